# revision 3
# baseline (speedup 1.0000x reference)
"""Dual-branch cross-attention block (nn_Attention) on 8 Trainium2 NeuronCores.

Sharding: pure data-parallel over batch B=8 - one batch element per core, no
collectives. Each core runs the full block for its element:
  QKV projections (fp32r matmuls, weights streamed from HBM),
  4 attention patterns x 12 heads (scores computed transposed, softmax via
  exp + ones-column sumexp + reciprocal outer-product broadcast),
  concat-FC and output projections, biases injected as K=1 outer-product
  matmul PSUM inits.

All matmuls run in float32r (fp32 storage, RNE-rounded to 11 explicit
mantissa bits on the multiply path) which streams at 1 cycle/row vs 4 for
plain fp32. Weights/biases are pre-rounded host-side so they can be DMA'd
straight into float32r tiles with no on-chip conversion pass.

SBUF is tight (~201 KB/partition usable): branch-p attention -> FC -> out
runs before branch-m so the two concat buffers share one slot, Wfc streams
as two [768,768] halves through the same pool as the square weights, and
x/mol staging + biases go through small recycling pools.
"""

import numpy as np

import concourse.bass as bass
import concourse.mybir as mybir
import concourse.tile as tile
from concourse import bacc
from concourse.masks import make_identity
from concourse.bass_utils import run_bass_kernel_spmd

F32 = mybir.dt.float32
F32R = mybir.dt.float32r
AF = mybir.ActivationFunctionType

B, S, D, H, DH = 8, 512, 768, 12, 64
KT = D // 128          # 6 k-tiles over D
FCKT = 2 * D // 128    # 12 k-tiles over 2D
ST = S // 128          # 4 s-tiles


def rne_fp32r(a: np.ndarray) -> np.ndarray:
    """Round-to-nearest-even to 11 explicit mantissa bits (hw fp32r rounding)."""
    u = np.ascontiguousarray(a, dtype=np.float32).view(np.uint32).astype(np.uint64)
    lsb = (u >> np.uint64(12)) & np.uint64(1)
    r = (u + np.uint64(0x7FF) + lsb) & np.uint64(0xFFFFF000)
    return r.astype(np.uint32).view(np.float32)


WEIGHT_NAMES = ["Wq", "Wk", "Wv", "Wqm", "Wkm", "Wvm", "Wfc", "Wfcm", "Wo", "Wom"]
BIAS_NAMES = ["bq", "bk", "bv", "bqm", "bkm", "bvm", "bfc", "bfcm", "bo", "bom"]


def build_program():
    nc = bacc.Bacc("TRN2", target_bir_lowering=False, debug=False, num_devices=8)

    x_h = nc.dram_tensor("x_h", [S, D], F32, kind="ExternalInput")
    x_m = nc.dram_tensor("x_m", [S, D], F32, kind="ExternalInput")
    wd = {
        n: nc.dram_tensor(n, [2 * D if n in ("Wfc", "Wfcm") else D, D], F32R,
                          kind="ExternalInput")
        for n in WEIGHT_NAMES
    }
    bd = {n: nc.dram_tensor(n, [1, D], F32R, kind="ExternalInput") for n in BIAS_NAMES}
    out_p = nc.dram_tensor("out_p", [S, D], F32, kind="ExternalOutput")
    out_m = nc.dram_tensor("out_m", [S, D], F32, kind="ExternalOutput")

    with tile.TileContext(nc) as tc:
        with tc.tile_pool(name="cst", bufs=1) as cst, \
             tc.tile_pool(name="persist", bufs=1) as pp, \
             tc.tile_pool(name="xfc", bufs=2) as xfcp, \
             tc.tile_pool(name="aTpool", bufs=1) as atp, \
             tc.tile_pool(name="w768", bufs=2) as wp, \
             tc.tile_pool(name="xn", bufs=2) as xnp, \
             tc.tile_pool(name="bias", bufs=2) as biasp, \
             tc.tile_pool(name="et", bufs=5) as etp, \
             tc.tile_pool(name="scratch", bufs=6) as scr, \
             tc.tile_pool(name="psA", bufs=2, space="PSUM") as psA, \
             tc.tile_pool(name="psS", bufs=3, space="PSUM") as psS, \
             tc.tile_pool(name="psV", bufs=2, space="PSUM") as psV:

            # ---------------- constants ----------------
            ident = cst.tile([128, 128], F32)
            make_identity(nc, ident[:])
            ones_f = scr.tile([1, 768], F32, tag="scratch")
            nc.vector.memset(ones_f[:], 1.0)
            ones = cst.tile([1, 768], F32R)
            nc.vector.tensor_copy(out=ones[:], in_=ones_f[:])
            onescol_f = scr.tile([128, 1], F32, tag="scratch")
            nc.vector.memset(onescol_f[:], 1.0)
            onescol = cst.tile([128, 1], F32R)
            nc.vector.tensor_copy(out=onescol[:], in_=onescol_f[:])

            def bias_row(n):
                t = biasp.tile([1, D], F32R, tag="bias")
                nc.sync.dma_start(t[:], bd[n][:])
                return t

            # ------------- load inputs, transpose to [D, S] -------------
            def load_transposed(x_dram, tag):
                xt = xfcp.tile([128, KT, S], F32R, tag="xfc")
                for st in range(ST):
                    xs = xnp.tile([128, D], F32, tag="xn")
                    nc.sync.dma_start(xs[:], x_dram[st * 128:(st + 1) * 128, :])
                    for dt in range(KT):
                        pt = psA.tile([128, 512], F32, tag="proj")
                        nc.tensor.transpose(pt[:, :128], xs[:, dt * 128:(dt + 1) * 128],
                                            ident[:])
                        nc.vector.tensor_copy(out=xt[:, dt, st * 128:(st + 1) * 128],
                                              in_=pt[:, :128])
                return xt

            xt = load_transposed(x_h, "xh")
            mt = load_transposed(x_m, "xm")

            def load_w768(dram_slice):
                t = wp.tile([128, KT, D], F32R, tag="w768")
                nc.sync.dma_start(t[:], dram_slice.rearrange("(ko ki) m -> ki ko m", ki=128))
                return t

            # ------------- transposed projection: yT = W.T @ xT + b -------------
            def proj_T(wname, bname, src_t, tag):
                w = load_w768(wd[wname][:, :])
                brow = bias_row(bname)
                yt = pp.tile([128, KT, S], F32R, tag=tag)
                for m in range(KT):
                    pt = psA.tile([128, 512], F32, tag="proj")
                    nc.tensor.matmul(pt[:], brow[:, m * 128:(m + 1) * 128],
                                     ones[:, :512], start=True, stop=False)
                    for k in range(KT):
                        nc.tensor.matmul(pt[:], w[:, k, m * 128:(m + 1) * 128],
                                         src_t[:, k, :], start=False, stop=(k == KT - 1))
                    nc.scalar.activation(yt[:, m, :], pt[:], AF.Copy)
                return yt

            # ------------- normal-layout V projection with ones column -------------
            def proj_vaug(wname, bname, src_t, tag):
                w = load_w768(wd[wname][:, :])
                brow = bias_row(bname)
                va = pp.tile([128, ST, H, DH + 1], F32R, tag=tag)
                for st in range(ST):
                    for c in range(2):  # two 384-wide chunks -> heads 6c..6c+5
                        pt = psA.tile([128, 512], F32, tag="proj")
                        nc.tensor.matmul(pt[:, :384], ones[:, :128],
                                         brow[:, c * 384:(c + 1) * 384],
                                         start=True, stop=False)
                        for k in range(KT):
                            nc.tensor.matmul(pt[:, :384],
                                             src_t[:, k, st * 128:(st + 1) * 128],
                                             w[:, k, c * 384:(c + 1) * 384],
                                             start=False, stop=(k == KT - 1))
                        nc.vector.tensor_copy(
                            out=va[:, st, c * 6:(c + 1) * 6, 0:DH],
                            in_=pt[:, :384].rearrange("p (h d) -> p h d", d=DH))
                    nc.vector.tensor_copy(
                        out=va[:, st, :, DH:DH + 1],
                        in_=onescol[:, None, :].broadcast_to([128, H, 1]))
                return va

            qt = proj_T("Wq", "bq", xt, "qt")
            kt = proj_T("Wk", "bk", xt, "kt")
            vaug = proj_vaug("Wv", "bv", xt, "vaug")
            qmt = proj_T("Wqm", "bqm", mt, "qmt")
            kmt = proj_T("Wkm", "bkm", mt, "kmt")
            vmaug = proj_vaug("Wvm", "bvm", mt, "vmaug")

            # ------------- attention: one (pattern, head) unit -------------
            def attn_unit(h, q_src, k_src, v_src, dst, half):
                b0 = (h % 2) * 64
                ko = h // 2
                ets = []
                for i in range(ST):
                    stp = psS.tile([128, 512], F32, tag="st")
                    nc.tensor.matmul(stp[:], k_src[b0:b0 + 64, ko, i * 128:(i + 1) * 128],
                                     q_src[b0:b0 + 64, ko, :], start=True, stop=True)
                    et = etp.tile([128, 512], F32R, tag="et")
                    nc.scalar.activation(et[:], stp[:], AF.Exp, scale=1.0 / 8.0)
                    ets.append(et)
                avp = psV.tile([DH + 1, 512], F32, tag="av")
                for i in range(ST):
                    nc.tensor.matmul(avp[:], v_src[:, i, h, :], ets[i][:],
                                     start=(i == 0), stop=(i == ST - 1))
                t65 = scr.tile([DH + 1, 512], F32, tag="scratch")
                nc.scalar.activation(t65[:], avp[:], AF.Copy)
                recip = scr.tile([1, 512], F32R, tag="scratch")
                with nc.allow_low_precision(reason="softmax recip feeds fp32r matmul"):
                    nc.vector.reciprocal(recip[:], t65[DH:DH + 1, :])
                bcp = psS.tile([64, 512], F32, tag="st")
                nc.tensor.matmul(bcp[:], ones[:, :64], recip[:], start=True, stop=True)
                nc.vector.tensor_mul(dst[b0:b0 + 64, half * 6 + ko, :],
                                     t65[0:DH, :], bcp[:])

            # ------------- fc + out projection for one branch -------------
            def fc_out(wfc_name, bfc_name, wo_name, bo_name, aT, out_dram):
                wfcA = load_w768(wd[wfc_name][0:D, :])
                wfcB = load_w768(wd[wfc_name][D:2 * D, :])
                bfc = bias_row(bfc_name)
                fcT = xfcp.tile([128, KT, S], F32R, tag="xfc")
                for m in range(KT):
                    pt = psA.tile([128, 512], F32, tag="proj")
                    nc.tensor.matmul(pt[:], bfc[:, m * 128:(m + 1) * 128],
                                     ones[:, :512], start=True, stop=False)
                    for k in range(FCKT):
                        w = wfcA if k < KT else wfcB
                        nc.tensor.matmul(pt[:], w[:, k % KT, m * 128:(m + 1) * 128],
                                         aT[:, k, :], start=False, stop=(k == FCKT - 1))
                    nc.scalar.activation(fcT[:, m, :], pt[:], AF.Copy)
                wo = load_w768(wd[wo_name][:, :])
                bo = bias_row(bo_name)
                for st in range(ST):
                    for c0, cw in ((0, 512), (512, 256)):
                        pt = psA.tile([128, 512], F32, tag="proj")
                        nc.tensor.matmul(pt[:, :cw], ones[:, :128],
                                         bo[:, c0:c0 + cw], start=True, stop=False)
                        for k in range(KT):
                            nc.tensor.matmul(pt[:, :cw],
                                             fcT[:, k, st * 128:(st + 1) * 128],
                                             wo[:, k, c0:c0 + cw],
                                             start=False, stop=(k == KT - 1))
                        ot = scr.tile([128, 512], F32, tag="scratch")
                        nc.vector.tensor_copy(out=ot[:, :cw], in_=pt[:, :cw])
                        nc.sync.dma_start(out_dram[st * 128:(st + 1) * 128, c0:c0 + cw],
                                          ot[:, :cw])

            # branch p: a_pp | a_mp -> fc -> out   (aT slot then recycled for m)
            aTp = atp.tile([128, FCKT, S], F32R, tag="aT")
            for h in range(H):
                attn_unit(h, qt, kt, vaug, aTp, 0)    # a_pp
            for h in range(H):
                attn_unit(h, qmt, kt, vaug, aTp, 1)   # a_mp
            fc_out("Wfc", "bfc", "Wo", "bo", aTp, out_p)

            # branch m: a_mm | a_pm -> fc -> out
            aTm = atp.tile([128, FCKT, S], F32R, tag="aT")
            for h in range(H):
                attn_unit(h, qmt, kmt, vmaug, aTm, 0)  # a_mm
            for h in range(H):
                attn_unit(h, qt, kmt, vmaug, aTm, 1)   # a_pm
            fc_out("Wfcm", "bfcm", "Wom", "bom", aTm, out_m)

    nc.compile()
    return nc


_PROGRAM_CACHE = {}


def kernel(hidden_states, mol, Wq, bq, Wk, bk, Wv, bv, Wqm, bqm, Wkm, bkm,
           Wvm, bvm, Wfc, bfc, Wfcm, bfcm, Wo, bo, Wom, bom):
    if "nc" not in _PROGRAM_CACHE:
        _PROGRAM_CACHE["nc"] = build_program()
    nc = _PROGRAM_CACHE["nc"]

    weights = {"Wq": Wq, "Wk": Wk, "Wv": Wv, "Wqm": Wqm, "Wkm": Wkm, "Wvm": Wvm,
               "Wfc": Wfc, "Wfcm": Wfcm, "Wo": Wo, "Wom": Wom}
    biases = {"bq": bq, "bk": bk, "bv": bv, "bqm": bqm, "bkm": bkm, "bvm": bvm,
              "bfc": bfc, "bfcm": bfcm, "bo": bo, "bom": bom}
    shared = {n: rne_fp32r(w) for n, w in weights.items()}
    shared.update({n: rne_fp32r(np.asarray(b, np.float32).reshape(1, D))
                   for n, b in biases.items()})

    hidden_states = np.ascontiguousarray(hidden_states, dtype=np.float32)
    mol = np.ascontiguousarray(mol, dtype=np.float32)
    in_maps = [dict(shared,
                    x_h=np.ascontiguousarray(hidden_states[b]),
                    x_m=np.ascontiguousarray(mol[b])) for b in range(B)]

    res = run_bass_kernel_spmd(nc, in_maps, core_ids=list(range(B)))
    attn_prot = np.stack([res.results[b]["out_p"] for b in range(B)])
    attn_mol = np.stack([res.results[b]["out_m"] for b in range(B)])
    return attn_prot, attn_mol


# revision 18
# speedup vs baseline: 312.2655x; 312.2655x over previous
"""Dual-branch cross-attention block (nn_Attention) on 8 Trainium2 NeuronCores.

Sharding: pure data-parallel over batch B=8 - one batch element per core, no
collectives. Each core runs the full block for its element:
  QKV projections (fp32r matmuls, weights streamed from HBM),
  4 attention patterns x 12 heads (scores computed transposed, softmax via
  exp + ones-column sumexp + reciprocal outer-product broadcast),
  concat-FC and output projections, biases injected as K=1 outer-product
  matmul PSUM inits.

All matmuls run in float32r (fp32 storage, RNE-rounded to 11 explicit
mantissa bits on the multiply path) which streams at 1 cycle/row vs 4 for
plain fp32. Weights/biases are pre-rounded host-side so they can be DMA'd
straight into float32r tiles with no on-chip conversion pass.

SBUF is tight (~201 KB/partition usable): branch-p attention -> FC -> out
runs before branch-m so the two concat buffers share one slot, Wfc streams
as two [768,768] halves through the same pool as the square weights, and
x/mol staging + biases go through small recycling pools.
"""

import numpy as np

import concourse.bass as bass
import concourse.mybir as mybir
import concourse.tile as tile
from concourse import bacc
from concourse.masks import make_identity
from concourse.bass_utils import run_bass_kernel_spmd

F32 = mybir.dt.float32
F32R = mybir.dt.float32r
BF16 = mybir.dt.bfloat16
AF = mybir.ActivationFunctionType

B, S, D, H, DH = 8, 512, 768, 12, 64
KT = D // 128          # 6 k-tiles over D
FCKT = 2 * D // 128    # 12 k-tiles over 2D
ST = S // 128          # 4 s-tiles


def rne_fp32r(a: np.ndarray) -> np.ndarray:
    """Round-to-nearest-even to 11 explicit mantissa bits (hw fp32r rounding)."""
    u = np.ascontiguousarray(a, dtype=np.float32).view(np.uint32).astype(np.uint64)
    lsb = (u >> np.uint64(12)) & np.uint64(1)
    r = (u + np.uint64(0x7FF) + lsb) & np.uint64(0xFFFFF000)
    return r.astype(np.uint32).view(np.float32)


WEIGHT_NAMES = ["Wq", "Wk", "Wv", "Wqm", "Wkm", "Wvm", "Wfc", "Wfcm", "Wo", "Wom"]
BIAS_NAMES = ["bq", "bk", "bv", "bqm", "bkm", "bvm", "bfc", "bfcm", "bo", "bom"]


def build_program():
    nc = bacc.Bacc("TRN2", target_bir_lowering=False, debug=False, num_devices=8)

    x_h = nc.dram_tensor("x_h", [S, D], F32, kind="ExternalInput")
    x_m = nc.dram_tensor("x_m", [S, D], F32, kind="ExternalInput")
    wd = {
        n: nc.dram_tensor(n, [2 * D if n in ("Wfc", "Wfcm") else D, D],
                          BF16 if n in ("Wfc", "Wfcm") else F32R,
                          kind="ExternalInput")
        for n in WEIGHT_NAMES
    }
    bd = {n: nc.dram_tensor(n, [1, D], F32R, kind="ExternalInput") for n in BIAS_NAMES}
    out_p = nc.dram_tensor("out_p", [S, D], F32, kind="ExternalOutput")
    out_m = nc.dram_tensor("out_m", [S, D], F32, kind="ExternalOutput")

    with tile.TileContext(nc) as tc:
        with tc.tile_pool(name="cst", bufs=1) as cst, \
             tc.tile_pool(name="persist", bufs=1) as pp, \
             tc.tile_pool(name="xfc", bufs=2) as xfcp, \
             tc.tile_pool(name="aTpool", bufs=2) as atp, \
             tc.tile_pool(name="w768", bufs=2) as wp, \
             tc.tile_pool(name="xn", bufs=2) as xnp, \
             tc.tile_pool(name="bias", bufs=2) as biasp, \
             tc.tile_pool(name="et", bufs=8) as etp, \
             tc.tile_pool(name="scratch", bufs=8) as scr, \
             tc.tile_pool(name="psA", bufs=2, space="PSUM") as psA, \
             tc.tile_pool(name="psS", bufs=3, space="PSUM") as psS, \
             tc.tile_pool(name="psV", bufs=3, space="PSUM") as psV:

            # ---------------- constants ----------------
            ident = cst.tile([128, 128], F32)
            make_identity(nc, ident[:])
            ones_f = biasp.tile([1, 768], F32, tag="bias")
            nc.vector.memset(ones_f[:], 1.0)
            ones = cst.tile([1, 768], F32R)
            nc.vector.tensor_copy(out=ones[:], in_=ones_f[:])
            onescol_f = biasp.tile([128, 1], F32, tag="bias")
            nc.vector.memset(onescol_f[:], 1.0)
            onescol = cst.tile([128, 1], F32R)
            nc.vector.tensor_copy(out=onescol[:], in_=onescol_f[:])

            def bias_row(n):
                t = biasp.tile([1, D], F32R, tag="bias")
                nc.sync.dma_start(t[:], bd[n][:])
                return t

            def bias_col(n):
                t = biasp.tile([128, KT], F32, tag="bias")
                nc.sync.dma_start(
                    t[:], bd[n].bitcast(F32).rearrange("one (m p) -> (one p) m", p=128))
                return t

            # ------------- load inputs, transpose to [D, S] -------------
            def load_transposed(x_dram, tag):
                xt = xfcp.tile([128, KT, S], F32R, tag="xfc")
                for st in range(ST):
                    xs = xnp.tile([128, D], F32, tag="xn")
                    nc.sync.dma_start(xs[:], x_dram[st * 128:(st + 1) * 128, :])
                    for dt in range(KT):
                        pt = psA.tile([128, 512], F32, tag="proj")
                        nc.tensor.transpose(pt[:, :128], xs[:, dt * 128:(dt + 1) * 128],
                                            ident[:])
                        nc.vector.tensor_copy(out=xt[:, dt, st * 128:(st + 1) * 128],
                                              in_=pt[:, :128])
                return xt

            def load_w768(dram_slice, dtype=F32R):
                # one DMA per contraction k-tile: contiguous 384KB runs, and
                # consumers can start as soon as k-tile 0 lands
                t = wp.tile([128, KT, D], dtype, tag="w768")
                src3 = dram_slice.rearrange("(ko ki) m -> ki ko m", ki=128)
                for k in range(KT):
                    nc.sync.dma_start(t[:, k, :], src3[:, k, :])
                return t

            xt = load_transposed(x_h, "xh")
            wq_t = load_w768(wd["Wq"][:, :])
            mt = load_transposed(x_m, "xm")

            # ------------- transposed projection: yT = W.T @ xT + b -------------
            def proj_T(wname, bname, src_t, tag, w=None):
                if w is None:
                    w = load_w768(wd[wname][:, :])
                bcol = bias_col(bname)
                yt = pp.tile([128, KT, S], F32R, tag=tag)
                for m in range(KT):
                    pt = psA.tile([128, 512], F32, tag="proj")
                    for k in range(KT):
                        nc.tensor.matmul(pt[:], w[:, k, m * 128:(m + 1) * 128],
                                         src_t[:, k, :], start=(k == 0), stop=(k == KT - 1))
                    nc.vector.tensor_scalar_add(out=yt[:, m, :], in0=pt[:],
                                                scalar1=bcol[:, m:m + 1])
                return yt

            # ------------- normal-layout V projection with ones column -------------
            def proj_vaug(wname, bname, src_t, tag):
                w = load_w768(wd[wname][:, :])
                brow = bias_row(bname)
                va = pp.tile([128, ST, H, DH + 1], F32R, tag=tag)
                for st in range(ST):
                    for c in range(2):  # two 384-wide chunks -> heads 6c..6c+5
                        pt = psA.tile([128, 512], F32, tag="proj")
                        nc.tensor.matmul(pt[:, :384], ones[:, :128],
                                         brow[:, c * 384:(c + 1) * 384],
                                         start=True, stop=False)
                        for k in range(KT):
                            nc.tensor.matmul(pt[:, :384],
                                             src_t[:, k, st * 128:(st + 1) * 128],
                                             w[:, k, c * 384:(c + 1) * 384],
                                             start=False, stop=(k == KT - 1))
                        nc.vector.tensor_copy(
                            out=va[:, st, c * 6:(c + 1) * 6, 0:DH],
                            in_=pt[:, :384].rearrange("p (h d) -> p h d", d=DH))
                    nc.vector.tensor_copy(
                        out=va[:, st, :, DH:DH + 1],
                        in_=onescol[:, None, :].broadcast_to([128, H, 1]))
                return va

            qt = proj_T("Wq", "bq", xt, "qt", w=wq_t)
            kt = proj_T("Wk", "bk", xt, "kt")
            vaug = proj_vaug("Wv", "bv", xt, "vaug")
            qmt = proj_T("Wqm", "bqm", mt, "qmt")
            kmt = proj_T("Wkm", "bkm", mt, "kmt")
            vmaug = proj_vaug("Wvm", "bvm", mt, "vmaug")

            # ------------- attention: one (pattern, head) unit -------------
            def attn_unit(h, q_src, k_src, v_src, dst, half):
                b0 = (h % 2) * 64
                ko = h // 2
                ets = []
                for i in range(ST):
                    stp = psS.tile([128, 512], F32, tag="st")
                    nc.tensor.matmul(stp[:], k_src[b0:b0 + 64, ko, i * 128:(i + 1) * 128],
                                     q_src[b0:b0 + 64, ko, :], start=True, stop=True)
                    et = etp.tile([128, 512], F32R, tag="et")
                    nc.scalar.activation(et[:], stp[:], AF.Exp, scale=1.0 / 8.0)
                    ets.append(et)
                avp = psV.tile([DH + 1, 512], F32, tag="av")
                for i in range(ST):
                    nc.tensor.matmul(avp[:], v_src[:, i, h, :], ets[i][:],
                                     start=(i == 0), stop=(i == ST - 1))
                recip = scr.tile([1, 512], F32R, tag="scratch")
                with nc.allow_low_precision(reason="softmax recip feeds fp32r matmul"):
                    nc.vector.reciprocal(recip[:], avp[DH:DH + 1, :])
                t64 = scr.tile([DH, 512], F32, tag="scratch")
                nc.vector.tensor_copy(out=t64[:], in_=avp[0:DH, :])
                bcp = psV.tile([64, 512], F32, tag="av")
                nc.tensor.matmul(bcp[:], ones[:, :64], recip[:], start=True, stop=True)
                nc.vector.tensor_mul(dst[b0:b0 + 64, half * 6 + ko, :],
                                     t64[:], bcp[:])

            # ------------- fc + out projection for one branch -------------
            def fc_out(wfc_name, bfc_name, wo_name, bo_name, aT, out_dram):
                wfcA = load_w768(wd[wfc_name][0:D, :], dtype=BF16)
                wfcB = load_w768(wd[wfc_name][D:2 * D, :], dtype=BF16)
                bfcc = bias_col(bfc_name)
                fcT = xfcp.tile([128, KT, S], F32R, tag="xfc")
                for m in range(KT):
                    pt = psA.tile([128, 512], F32, tag="proj")
                    for k in range(FCKT):
                        w = wfcA if k < KT else wfcB
                        nc.tensor.matmul(pt[:], w[:, k % KT, m * 128:(m + 1) * 128],
                                         aT[:, k, :], start=(k == 0), stop=(k == FCKT - 1))
                    nc.vector.tensor_scalar_add(out=fcT[:, m, :], in0=pt[:],
                                                scalar1=bfcc[:, m:m + 1])
                wo = load_w768(wd[wo_name][:, :])
                bo = bias_row(bo_name)
                for st in range(ST):
                    for c0, cw in ((0, 512), (512, 256)):
                        pt = psA.tile([128, 512], F32, tag="proj")
                        nc.tensor.matmul(pt[:, :cw], ones[:, :128],
                                         bo[:, c0:c0 + cw], start=True, stop=False)
                        for k in range(KT):
                            nc.tensor.matmul(pt[:, :cw],
                                             fcT[:, k, st * 128:(st + 1) * 128],
                                             wo[:, k, c0:c0 + cw],
                                             start=False, stop=(k == KT - 1))
                        ot = scr.tile([128, 512], F32, tag="scratch")
                        nc.vector.tensor_copy(out=ot[:, :cw], in_=pt[:, :cw])
                        nc.sync.dma_start(out_dram[st * 128:(st + 1) * 128, c0:c0 + cw],
                                          ot[:, :cw])

            # branch p: a_pp | a_mp -> fc -> out   (aT slot then recycled for m)
            aTp = atp.tile([128, FCKT, S], BF16, tag="aT")
            for h in range(H):
                attn_unit(h, qt, kt, vaug, aTp, 0)    # a_pp
            for h in range(H):
                attn_unit(h, qmt, kt, vaug, aTp, 1)   # a_mp
            fc_out("Wfc", "bfc", "Wo", "bo", aTp, out_p)

            # branch m: a_mm | a_pm -> fc -> out
            aTm = atp.tile([128, FCKT, S], BF16, tag="aT")
            for h in range(H):
                attn_unit(h, qmt, kmt, vmaug, aTm, 0)  # a_mm
            for h in range(H):
                attn_unit(h, qt, kmt, vmaug, aTm, 1)   # a_pm
            fc_out("Wfcm", "bfcm", "Wom", "bom", aTm, out_m)

    nc.compile()
    return nc


_PROGRAM_CACHE = {}


def prepare_in_maps(inputs):
    """Full-input dict -> per-core in_maps with host-side dtype prep."""
    import ml_dtypes
    shared = {}
    for n in WEIGHT_NAMES:
        if n in ("Wfc", "Wfcm"):
            shared[n] = np.asarray(inputs[n], np.float32).astype(ml_dtypes.bfloat16)
        else:
            shared[n] = rne_fp32r(inputs[n])
    for n in BIAS_NAMES:
        shared[n] = rne_fp32r(np.asarray(inputs[n], np.float32).reshape(1, D))
    hs = np.ascontiguousarray(inputs["hidden_states"], dtype=np.float32)
    ml = np.ascontiguousarray(inputs["mol"], dtype=np.float32)
    return [dict(shared, x_h=np.ascontiguousarray(hs[b]),
                 x_m=np.ascontiguousarray(ml[b])) for b in range(B)]


def kernel(hidden_states, mol, Wq, bq, Wk, bk, Wv, bv, Wqm, bqm, Wkm, bkm,
           Wvm, bvm, Wfc, bfc, Wfcm, bfcm, Wo, bo, Wom, bom):
    if "nc" not in _PROGRAM_CACHE:
        _PROGRAM_CACHE["nc"] = build_program()
    nc = _PROGRAM_CACHE["nc"]
    in_maps = prepare_in_maps(dict(
        hidden_states=hidden_states, mol=mol, Wq=Wq, bq=bq, Wk=Wk, bk=bk,
        Wv=Wv, bv=bv, Wqm=Wqm, bqm=bqm, Wkm=Wkm, bkm=bkm, Wvm=Wvm, bvm=bvm,
        Wfc=Wfc, bfc=bfc, Wfcm=Wfcm, bfcm=bfcm, Wo=Wo, bo=bo, Wom=Wom, bom=bom))

    res = run_bass_kernel_spmd(nc, in_maps, core_ids=list(range(B)))
    attn_prot = np.stack([res.results[b]["out_p"] for b in range(B)])
    attn_mol = np.stack([res.results[b]["out_m"] for b in range(B)])
    return attn_prot, attn_mol


# revision 20
# speedup vs baseline: 324.4992x; 1.0392x over previous
"""Dual-branch cross-attention block (nn_Attention) on 8 Trainium2 NeuronCores.

Sharding: pure data-parallel over batch B=8 - one batch element per core, no
collectives. Each core runs the full block for its element:
  QKV projections (fp32r matmuls, weights streamed from HBM),
  4 attention patterns x 12 heads (scores computed transposed, softmax via
  exp + ones-column sumexp + reciprocal outer-product broadcast),
  concat-FC and output projections, biases injected as K=1 outer-product
  matmul PSUM inits.

All matmuls run in float32r (fp32 storage, RNE-rounded to 11 explicit
mantissa bits on the multiply path) which streams at 1 cycle/row vs 4 for
plain fp32. Weights/biases are pre-rounded host-side so they can be DMA'd
straight into float32r tiles with no on-chip conversion pass.

SBUF is tight (~201 KB/partition usable): branch-p attention -> FC -> out
runs before branch-m so the two concat buffers share one slot, Wfc streams
as two [768,768] halves through the same pool as the square weights, and
x/mol staging + biases go through small recycling pools.
"""

import numpy as np

import concourse.bass as bass
import concourse.mybir as mybir
import concourse.tile as tile
from concourse import bacc
from concourse.masks import make_identity
from concourse.bass_utils import run_bass_kernel_spmd

F32 = mybir.dt.float32
F32R = mybir.dt.float32r
BF16 = mybir.dt.bfloat16
AF = mybir.ActivationFunctionType

B, S, D, H, DH = 8, 512, 768, 12, 64
KT = D // 128          # 6 k-tiles over D
FCKT = 2 * D // 128    # 12 k-tiles over 2D
ST = S // 128          # 4 s-tiles


def rne_fp32r(a: np.ndarray) -> np.ndarray:
    """Round-to-nearest-even to 11 explicit mantissa bits (hw fp32r rounding)."""
    u = np.ascontiguousarray(a, dtype=np.float32).view(np.uint32).astype(np.uint64)
    lsb = (u >> np.uint64(12)) & np.uint64(1)
    r = (u + np.uint64(0x7FF) + lsb) & np.uint64(0xFFFFF000)
    return r.astype(np.uint32).view(np.float32)


WEIGHT_NAMES = ["Wq", "Wk", "Wv", "Wqm", "Wkm", "Wvm", "Wfc", "Wfcm", "Wo", "Wom"]
BIAS_NAMES = ["bq", "bk", "bv", "bqm", "bkm", "bvm", "bfc", "bfcm", "bo", "bom"]


def build_program():
    nc = bacc.Bacc("TRN2", target_bir_lowering=False, debug=False, num_devices=8)

    x_h = nc.dram_tensor("x_h", [S, D], F32, kind="ExternalInput")
    x_m = nc.dram_tensor("x_m", [S, D], F32, kind="ExternalInput")
    wd = {
        n: nc.dram_tensor(n, [2 * D if n in ("Wfc", "Wfcm") else D, D],
                          BF16 if n in ("Wfc", "Wfcm") else F32R,
                          kind="ExternalInput")
        for n in WEIGHT_NAMES
    }
    bd = {n: nc.dram_tensor(n, [1, D], F32R, kind="ExternalInput") for n in BIAS_NAMES}
    out_p = nc.dram_tensor("out_p", [S, D], F32, kind="ExternalOutput")
    out_m = nc.dram_tensor("out_m", [S, D], F32, kind="ExternalOutput")

    with tile.TileContext(nc) as tc:
        with tc.tile_pool(name="cst", bufs=1) as cst, \
             tc.tile_pool(name="persist", bufs=1) as pp, \
             tc.tile_pool(name="xfc", bufs=2) as xfcp, \
             tc.tile_pool(name="aTpool", bufs=2) as atp, \
             tc.tile_pool(name="w768", bufs=2) as wp, \
             tc.tile_pool(name="xn", bufs=2) as xnp, \
             tc.tile_pool(name="bias", bufs=2) as biasp, \
             tc.tile_pool(name="et", bufs=8) as etp, \
             tc.tile_pool(name="scratch", bufs=8) as scr, \
             tc.tile_pool(name="psA", bufs=2, space="PSUM") as psA, \
             tc.tile_pool(name="psS", bufs=3, space="PSUM") as psS, \
             tc.tile_pool(name="psV", bufs=3, space="PSUM") as psV:

            # ---------------- constants ----------------
            ident = cst.tile([128, 128], F32)
            make_identity(nc, ident[:])
            ones_f = biasp.tile([1, 768], F32, tag="bias")
            nc.vector.memset(ones_f[:], 1.0)
            ones = cst.tile([1, 768], F32R)
            nc.vector.tensor_copy(out=ones[:], in_=ones_f[:])
            onescol_f = biasp.tile([128, 1], F32, tag="bias")
            nc.vector.memset(onescol_f[:], 1.0)
            onescol = cst.tile([128, 1], F32R)
            nc.vector.tensor_copy(out=onescol[:], in_=onescol_f[:])

            def bias_row(n):
                t = biasp.tile([1, D], F32R, tag="bias")
                nc.sync.dma_start(t[:], bd[n][:])
                return t

            def bias_col(n):
                t = biasp.tile([128, KT], F32, tag="bias")
                nc.sync.dma_start(
                    t[:], bd[n].bitcast(F32).rearrange("one (m p) -> (one p) m", p=128))
                return t

            # ------------- load inputs, transpose to [D, S] -------------
            def load_transposed(x_dram, tag):
                xt = xfcp.tile([128, KT, S], F32R, tag="xfc")
                for st in range(ST):
                    xs = xnp.tile([128, D], F32, tag="xn")
                    nc.sync.dma_start(xs[:], x_dram[st * 128:(st + 1) * 128, :])
                    for dt in range(KT):
                        pt = psA.tile([128, 512], F32, tag="proj")
                        nc.tensor.transpose(pt[:, :128], xs[:, dt * 128:(dt + 1) * 128],
                                            ident[:])
                        nc.vector.tensor_copy(out=xt[:, dt, st * 128:(st + 1) * 128],
                                              in_=pt[:, :128])
                return xt

            def load_w768(dram_slice, dtype=F32R):
                # one DMA per contraction k-tile: contiguous 384KB runs, and
                # consumers can start as soon as k-tile 0 lands
                t = wp.tile([128, KT, D], dtype, tag="w768")
                src3 = dram_slice.rearrange("(ko ki) m -> ki ko m", ki=128)
                for k in range(KT):
                    nc.sync.dma_start(t[:, k, :], src3[:, k, :])
                return t

            xt = load_transposed(x_h, "xh")
            wq_t = load_w768(wd["Wq"][:, :])
            mt = load_transposed(x_m, "xm")

            # ------------- transposed projection: yT = W.T @ xT + b -------------
            def proj_T(wname, bname, src_t, tag, w=None):
                if w is None:
                    w = load_w768(wd[wname][:, :])
                bcol = bias_col(bname)
                yt = pp.tile([128, KT, S], F32R, tag=tag)
                for m in range(KT):
                    pt = psA.tile([128, 512], F32, tag="proj")
                    for k in range(KT):
                        nc.tensor.matmul(pt[:], w[:, k, m * 128:(m + 1) * 128],
                                         src_t[:, k, :], start=(k == 0), stop=(k == KT - 1))
                    nc.vector.tensor_scalar_add(out=yt[:, m, :], in0=pt[:],
                                                scalar1=bcol[:, m:m + 1])
                return yt

            # ------------- normal-layout V projection with ones column -------------
            def proj_vaug(wname, bname, src_t, tag):
                w = load_w768(wd[wname][:, :])
                brow = bias_row(bname)
                va = pp.tile([128, ST, H, DH + 1], F32R, tag=tag)
                for st in range(ST):
                    for c in range(2):  # two 384-wide chunks -> heads 6c..6c+5
                        pt = psA.tile([128, 512], F32, tag="proj")
                        nc.tensor.matmul(pt[:, :384], ones[:, :128],
                                         brow[:, c * 384:(c + 1) * 384],
                                         start=True, stop=False)
                        for k in range(KT):
                            nc.tensor.matmul(pt[:, :384],
                                             src_t[:, k, st * 128:(st + 1) * 128],
                                             w[:, k, c * 384:(c + 1) * 384],
                                             start=False, stop=(k == KT - 1))
                        nc.vector.tensor_copy(
                            out=va[:, st, c * 6:(c + 1) * 6, 0:DH],
                            in_=pt[:, :384].rearrange("p (h d) -> p h d", d=DH))
                    nc.vector.tensor_copy(
                        out=va[:, st, :, DH:DH + 1],
                        in_=onescol[:, None, :].broadcast_to([128, H, 1]))
                return va

            qt = proj_T("Wq", "bq", xt, "qt", w=wq_t)
            kt = proj_T("Wk", "bk", xt, "kt")
            vaug = proj_vaug("Wv", "bv", xt, "vaug")
            qmt = proj_T("Wqm", "bqm", mt, "qmt")
            kmt = proj_T("Wkm", "bkm", mt, "kmt")
            vmaug = proj_vaug("Wvm", "bvm", mt, "vmaug")

            # ------------- attention: one (pattern, head) unit -------------
            def attn_unit(h, q_src, k_src, v_src, dst, half):
                b0 = (h % 2) * 64
                ko = h // 2
                ets = []
                for i in range(ST):
                    stp = psS.tile([128, 512], F32, tag="st")
                    nc.tensor.matmul(stp[:], k_src[b0:b0 + 64, ko, i * 128:(i + 1) * 128],
                                     q_src[b0:b0 + 64, ko, :], start=True, stop=True)
                    et = etp.tile([128, 512], F32R, tag="et")
                    nc.scalar.activation(et[:], stp[:], AF.Exp, scale=1.0 / 8.0)
                    ets.append(et)
                avp = psV.tile([DH + 1, 512], F32, tag="av")
                for i in range(ST):
                    nc.tensor.matmul(avp[:], v_src[:, i, h, :], ets[i][:],
                                     start=(i == 0), stop=(i == ST - 1))
                recip = scr.tile([1, 512], F32R, tag="scratch")
                with nc.allow_low_precision(reason="softmax recip feeds fp32r matmul"):
                    nc.vector.reciprocal(recip[:], avp[DH:DH + 1, :])
                t64 = scr.tile([DH, 512], F32, tag="scratch")
                nc.vector.tensor_copy(out=t64[:], in_=avp[0:DH, :])
                bcp = psV.tile([64, 512], F32, tag="av")
                nc.tensor.matmul(bcp[:], ones[:, :64], recip[:], start=True, stop=True)
                nc.vector.tensor_mul(dst[b0:b0 + 64, half * 6 + ko, :],
                                     t64[:], bcp[:])

            # ------------- fc + out projection for one branch -------------
            def fc_out(wfc_name, bfc_name, wo_name, bo_name, aT, out_dram):
                wfcA = load_w768(wd[wfc_name][0:D, :], dtype=BF16)
                wfcB = load_w768(wd[wfc_name][D:2 * D, :], dtype=BF16)
                bfcc = bias_col(bfc_name)
                fcT = xfcp.tile([128, KT, S], F32R, tag="xfc")
                for m in range(KT):
                    pt = psA.tile([128, 512], F32, tag="proj")
                    for k in range(FCKT):
                        w = wfcA if k < KT else wfcB
                        nc.tensor.matmul(pt[:], w[:, k % KT, m * 128:(m + 1) * 128],
                                         aT[:, k, :], start=(k == 0), stop=(k == FCKT - 1))
                    nc.vector.tensor_scalar_add(out=fcT[:, m, :], in0=pt[:],
                                                scalar1=bfcc[:, m:m + 1])
                wo = load_w768(wd[wo_name][:, :])
                bo = bias_row(bo_name)
                for st in range(ST):
                    for c0, cw in ((0, 512), (512, 256)):
                        pt = psA.tile([128, 512], F32, tag="proj")
                        nc.tensor.matmul(pt[:, :cw], ones[:, :128],
                                         bo[:, c0:c0 + cw], start=True, stop=False)
                        for k in range(KT):
                            nc.tensor.matmul(pt[:, :cw],
                                             fcT[:, k, st * 128:(st + 1) * 128],
                                             wo[:, k, c0:c0 + cw],
                                             start=False, stop=(k == KT - 1))
                        ot = scr.tile([128, 512], F32, tag="scratch")
                        nc.vector.tensor_copy(out=ot[:, :cw], in_=pt[:, :cw])
                        nc.sync.dma_start(out_dram[st * 128:(st + 1) * 128, c0:c0 + cw],
                                          ot[:, :cw])

            # branch p: a_pp | a_mp -> fc -> out   (aT slot then recycled for m)
            aTp = atp.tile([128, FCKT, S], BF16, tag="aT")
            for h in range(H):
                attn_unit(h, qt, kt, vaug, aTp, 0)    # a_pp
            for h in range(H):
                attn_unit(h, qmt, kt, vaug, aTp, 1)   # a_mp
            fc_out("Wfc", "bfc", "Wo", "bo", aTp, out_p)

            # branch m: a_mm | a_pm -> fc -> out
            aTm = atp.tile([128, FCKT, S], BF16, tag="aT")
            for h in range(H):
                attn_unit(h, qmt, kmt, vmaug, aTm, 0)  # a_mm
            for h in range(H):
                attn_unit(h, qt, kmt, vmaug, aTm, 1)   # a_pm
            fc_out("Wfcm", "bfcm", "Wom", "bom", aTm, out_m)

    nc.compile()
    return nc


_PROGRAM_CACHE = {}


def prepare_in_maps(inputs):
    """Full-input dict -> per-core in_maps with host-side dtype prep."""
    import ml_dtypes
    shared = {}
    for n in WEIGHT_NAMES:
        if n in ("Wfc", "Wfcm"):
            shared[n] = np.asarray(inputs[n], np.float32).astype(ml_dtypes.bfloat16)
        else:
            shared[n] = rne_fp32r(inputs[n])
    for n in BIAS_NAMES:
        shared[n] = rne_fp32r(np.asarray(inputs[n], np.float32).reshape(1, D))
    hs = np.ascontiguousarray(inputs["hidden_states"], dtype=np.float32)
    ml = np.ascontiguousarray(inputs["mol"], dtype=np.float32)
    return [dict(shared, x_h=np.ascontiguousarray(hs[b]),
                 x_m=np.ascontiguousarray(ml[b])) for b in range(B)]


def kernel(hidden_states, mol, Wq, bq, Wk, bk, Wv, bv, Wqm, bqm, Wkm, bkm,
           Wvm, bvm, Wfc, bfc, Wfcm, bfcm, Wo, bo, Wom, bom):
    if "nc" not in _PROGRAM_CACHE:
        _PROGRAM_CACHE["nc"] = build_program()
    nc = _PROGRAM_CACHE["nc"]
    in_maps = prepare_in_maps(dict(
        hidden_states=hidden_states, mol=mol, Wq=Wq, bq=bq, Wk=Wk, bk=bk,
        Wv=Wv, bv=bv, Wqm=Wqm, bqm=bqm, Wkm=Wkm, bkm=bkm, Wvm=Wvm, bvm=bvm,
        Wfc=Wfc, bfc=bfc, Wfcm=Wfcm, bfcm=bfcm, Wo=Wo, bo=bo, Wom=Wom, bom=bom))

    res = run_bass_kernel_spmd(nc, in_maps, core_ids=list(range(B)))
    attn_prot = np.stack([res.results[b]["out_p"] for b in range(B)])
    attn_mol = np.stack([res.results[b]["out_m"] for b in range(B)])
    return attn_prot, attn_mol
